# revision 11
# baseline (speedup 1.0000x reference)
"""Batched 2048-point DFT on 8 TRN2 NeuronCores — fp16 four-step, b-half pipelined.

n = 2048 = 128 * 16, m = 16*m1 + m2, k = k1 + 128*k2:
  X[b, k1 + 128*k2] = sum_m2 W16[m2,k2] * ( sum_m1 A_m2[m1,k1] * x[b, 16*m1+m2] )
with A_m2[m1,k1] = exp(-2i*pi*(16*m1+m2)*k1/2048).

All operands fp16 (tolerance 2e-2 >> fp16 error ~1e-3).  PSUM stays fp32.
The batch shard (512 rows/core) is processed in two halves of 256 so the
stage-2/3 work of half 0 overlaps stage 1 (and its input DMA) of half 1.

Layout tricks:
 - z col = b2*32 + q*2 + b0 (b = 2*b2+b0): stage-1 evacuation writes adjacent
   fp16 PAIRS (4B-aligned words, the fast strided path), transpose chunks are
   contiguous, and the permuted t-partition order is absorbed by a
   host-permuted block-diag S and the output unscramble (both free).
 - stage-1 re/im share one PSUM bank ([0:256]/[256:512]); only the first
   matmul uses start=True, later groups rely on the untouched has_written
   bits (skip_group_check) so the 'are' stationary is loaded once.
"""

import sys

for _p in ("/opt/trn_rl_repo", "/root/.axon_site/_ro/trn_rl_repo"):
    if _p not in sys.path:
        sys.path.insert(0, _p)

import numpy as np

import concourse.bass as bass
import concourse.mybir as mybir
import concourse.tile as tile
from concourse import bacc
from concourse.bass_utils import run_bass_kernel_spmd
from concourse.masks import make_identity

BATCH = 4096
NFFT = 2048
NCORES = 8
BPC = BATCH // NCORES  # 512
N1 = 128
N2 = 16
NH = 2
BH = BPC // NH  # 256 rows per half
NGRPH = BH * N2 // 128 // 4  # 8 stage-3 groups per half
NWARM = 7  # x4 matmuls to lift the HAM throttle during input DMA

F32 = mybir.dt.float32
F16 = mybir.dt.float16

_CACHE = {}


def _build_nc():
    nc = bacc.Bacc("TRN2", target_bir_lowering=False, debug=False)

    # xR layout: [m2 16, m1 128, b 512] flat [2048, 512]
    xre_d = nc.dram_tensor("xre", [N2 * N1, BPC], F16, kind="ExternalInput").ap()
    xim_d = nc.dram_tensor("xim", [N2 * N1, BPC], F16, kind="ExternalInput").ap()
    # A stationaries: [m2, m1 128, 3(re, im, imneg), k1 128] flat [2048, 384]
    a_d = nc.dram_tensor("amat", [N2 * N1, 3 * 128], F16, kind="ExternalInput").ap()
    # permuted S block-diag: [3, 128, 128]
    s_d = nc.dram_tensor("smat", [3 * 128, 128], F16, kind="ExternalInput").ap()
    # output dump: per (h, g): [128 (b2,kt,b0), 1024 (re|im, j, ko)]
    o_d = nc.dram_tensor(
        "odump", [NH * NGRPH * 128, 1024], F16, kind="ExternalOutput"
    ).ap()

    a_v = a_d.rearrange("(Q q p) (v k) -> Q p q v k", Q=4, q=4, v=3)
    s_v = s_d.rearrange("(v p) k -> v p k", v=3)
    o_v = o_d.rearrange("(g p) c -> g p c", g=NH * NGRPH)

    with tile.TileContext(nc) as tc:
        with (
            tc.tile_pool(name="const", bufs=1) as cpool,
            tc.tile_pool(name="x", bufs=1) as xpool,
            tc.tile_pool(name="z", bufs=1) as zpool,
            tc.tile_pool(name="t", bufs=3) as tpool,
            tc.tile_pool(name="o", bufs=3) as opool,
            tc.tile_pool(name="ps1", bufs=2, space="PSUM") as ps1pool,
            tc.tile_pool(name="pst", bufs=2, space="PSUM") as pstpool,
            tc.tile_pool(name="ps3", bufs=2, space="PSUM") as ps3pool,
        ):
            # identity first: used by PE warmup matmuls during the DMA fill
            ident = cpool.tile([128, 128], F16, tag="ident")
            make_identity(nc, ident[:])

            a_t = cpool.tile([128, N2, 3, 128], F16, tag="amat")
            xre_t = xpool.tile([128, N2, BPC], F16, tag="xre")
            xim_t = xpool.tile([128, N2, BPC], F16, tag="xim")
            for Q in range(4):
                qsl = slice(Q * 4, (Q + 1) * 4)
                nc.sync.dma_start(a_t[:, qsl], a_v[Q])
            for h in range(NH):
                hsl = slice(h * BH, (h + 1) * BH)
                xre_hv = xre_d[:, hsl].rearrange("(Q q p) b -> Q p q b", Q=4, q=4)
                xim_hv = xim_d[:, hsl].rearrange("(Q q p) b -> Q p q b", Q=4, q=4)
                for Q in range(4):
                    qsl = slice(Q * 4, (Q + 1) * 4)
                    nc.sync.dma_start(xre_t[:, qsl, hsl], xre_hv[Q])
                    nc.sync.dma_start(xim_t[:, qsl, hsl], xim_hv[Q])
            s_t = cpool.tile([128, 3, 128], F16, tag="smat")
            nc.sync.dma_start(s_t[:], s_v.transpose([1, 0, 2]))

            # HAM warmup: dead matmuls on the identity while inputs stream
            for _ in range(NWARM):
                psw = ps1pool.tile([128, 512], F32, tag="ps")
                for rep in range(4):
                    nc.tensor.matmul(
                        psw[:, rep * 128 : (rep + 1) * 128],
                        ident[:],
                        ident[:],
                        start=True,
                        stop=True,
                    )

            # stage-1 output: per half, col = b2*32 + q*2 + b0 (b = 2*b2+b0)
            z_re = zpool.tile([128, BPC * N2], F16, tag="zre")
            z_im = zpool.tile([128, BPC * N2], F16, tag="zim")
            z_re5 = z_re[:].rearrange("p (h c q b) -> p h c q b", h=NH, q=N2, b=2)
            z_im5 = z_im[:].rearrange("p (h c q b) -> p h c q b", h=NH, q=N2, b=2)

            def emit_s1(h, q):
                hsl = slice(h * BH, (h + 1) * BH)
                # separate banks for re/im so DVE and ScalarE never read the
                # same PSUM bank in parallel (half of each bank is unused)
                ps_re_t = ps1pool.tile([128, 512], F32, tag="ps")
                ps_im_t = ps1pool.tile([128, 512], F32, tag="ps")
                ps_re = ps_re_t[:, 0:BH]
                ps_im = ps_im_t[:, 0:BH]
                are = a_t[:, q, 0, :]
                aim = a_t[:, q, 1, :]
                aimn = a_t[:, q, 2, :]
                xr = xre_t[:, q, hsl]
                xi = xim_t[:, q, hsl]
                nc.tensor.matmul(ps_re, are, xr, start=True, stop=False)
                nc.tensor.matmul(ps_im, are, xi, start=True, stop=False)
                nc.tensor.matmul(ps_im, aim, xr, start=False, stop=True)
                nc.tensor.matmul(ps_re, aimn, xi, start=False, stop=True)
                nc.vector.tensor_copy(z_re5[:, h, :, q, :], ps_re)
                nc.scalar.copy(z_im5[:, h, :, q, :], ps_im)

            # ---- stage 2 + 3 ----
            sre = s_t[:, 0, :]
            sim = s_t[:, 1, :]
            simn = s_t[:, 2, :]

            def emit_tp(h, g):
                # 8 fp16 transposes into one [128,1024] fp16 PSUM bank
                pt = pstpool.tile([128, 1024], F16, tag="pt")
                for j in range(4):
                    c = (h * NGRPH + g) * 4 + j
                    csl = slice(c * 128, (c + 1) * 128)
                    nc.tensor.transpose(
                        pt[:, j * 128 : (j + 1) * 128], z_re[:, csl], ident[:]
                    )
                    nc.tensor.transpose(
                        pt[:, 512 + j * 128 : 512 + (j + 1) * 128],
                        z_im[:, csl],
                        ident[:],
                    )
                # single evacuation of the whole bank (fp16 2x on DVE)
                t_t = tpool.tile([128, 1024], F16, tag="t")
                if (h * NGRPH + g) % 8 < 5:
                    nc.vector.tensor_copy(t_t[:], pt[:])
                else:
                    nc.scalar.copy(t_t[:], pt[:])
                return t_t

            def emit_s3(h, g, t_t):
                t_re = t_t[:, 0:512]
                t_im = t_t[:, 512:1024]
                # re and im in one 2-bank tile -> single wide evacuation
                ps2 = ps3pool.tile([128, 1024], F32, tag="ps3")
                ps2_re = ps2[:, 0:512]
                ps2_im = ps2[:, 512:1024]
                nc.tensor.matmul(ps2_re, sre, t_re, start=True, stop=False)
                nc.tensor.matmul(ps2_im, sre, t_im, start=True, stop=False)
                nc.tensor.matmul(ps2_im, sim, t_re, start=False, stop=True)
                nc.tensor.matmul(ps2_re, simn, t_im, start=False, stop=True)
                o_t = opool.tile([128, 1024], F16, tag="o")
                if g % 2 == 0:
                    nc.vector.tensor_copy(o_t[:], ps2[:])
                else:
                    nc.scalar.copy(o_t[:], ps2[:])
                nc.sync.dma_start(o_v[h * NGRPH + g], o_t[:])

            # ---- emission schedule ----
            # s1(h0); then phase2(h0) with s1(h1) q-pairs woven between
            # groups; then phase2(h1).  Transposes run one group ahead.
            for q in range(N2):
                emit_s1(0, q)

            t_prev = emit_tp(0, 0)
            for g in range(NGRPH):
                if g + 1 < NGRPH:
                    t_next = emit_tp(0, g + 1)
                emit_s1(1, 2 * g)
                emit_s1(1, 2 * g + 1)
                emit_s3(0, g, t_prev)
                t_prev = t_next if g + 1 < NGRPH else None

            t_prev = emit_tp(1, 0)
            for g in range(NGRPH):
                t_next = emit_tp(1, g + 1) if g + 1 < NGRPH else None
                emit_s3(1, g, t_prev)
                t_prev = t_next

    nc.compile()
    return nc


def _consts():
    m1 = np.arange(N1, dtype=np.float64)
    k1 = np.arange(N1, dtype=np.float64)
    m2 = np.arange(N2, dtype=np.float64)
    k2 = np.arange(N2, dtype=np.float64)
    # A_m2[m1,k1] = exp(-2i pi (16 m1 + m2) k1 / 2048)
    a = np.empty((N2, 3, N1, N1), np.float16)
    for q in range(N2):
        ph = -2.0 * np.pi * np.outer(16.0 * m1 + q, k1) / NFFT
        a[q, 0] = np.cos(ph).astype(np.float16)
        a[q, 1] = np.sin(ph).astype(np.float16)
        a[q, 2] = -a[q, 1]
    # permuted block-diag S for the pair-interleaved t-partition order:
    # partition p = b2*32 + i*2 + b0 (bl = 2*b2+b0; i = m2 on rows, k2 on
    # cols); nonzero iff row bl == col bl
    ph16 = -2.0 * np.pi * np.outer(m2, k2) / N2
    w16re = np.cos(ph16).astype(np.float16)
    w16im = np.sin(ph16).astype(np.float16)
    p = np.arange(128)
    blp = (p // 32) * 2 + (p % 2)
    ip = (p % 32) // 2
    mask = (blp[:, None] == blp[None, :]).astype(np.float16)
    s = np.zeros((3, 128, 128), np.float16)
    s[0] = w16re[np.ix_(ip, ip)] * mask
    s[1] = w16im[np.ix_(ip, ip)] * mask
    s[2] = -s[1]
    return (
        np.ascontiguousarray(a.transpose(0, 2, 1, 3).reshape(N2 * 128, 3 * 128)),
        np.ascontiguousarray(s.reshape(3 * 128, 128)),
    )


def run(signal_re, signal_im, trace=False, tmpdir=None):
    if "nc" not in _CACHE:
        _CACHE["nc"] = _build_nc()
        _CACHE["c"] = _consts()
    nc = _CACHE["nc"]
    amat, smat = _CACHE["c"]

    sre = np.asarray(signal_re, dtype=np.float32).astype(np.float16)
    sim = np.asarray(signal_im, dtype=np.float32).astype(np.float16)

    in_maps = []
    for c in range(NCORES):
        bsl = slice(c * BPC, (c + 1) * BPC)
        # xR[m2, m1, b]
        xre = np.ascontiguousarray(
            sre[bsl].reshape(BPC, N1, N2).transpose(2, 1, 0).reshape(N2 * N1, BPC)
        )
        xim = np.ascontiguousarray(
            sim[bsl].reshape(BPC, N1, N2).transpose(2, 1, 0).reshape(N2 * N1, BPC)
        )
        in_maps.append({"xre": xre, "xim": xim, "amat": amat, "smat": smat})

    # first execution of a fresh NEFF occasionally fails with a transient
    # INTERNAL runtime error; retry a couple of times before giving up
    last_exc = None
    for attempt in range(3):
        try:
            br = run_bass_kernel_spmd(
                nc, in_maps, list(range(NCORES)), trace=trace, tmpdir=tmpdir
            )
            break
        except Exception as e:
            last_exc = e
            import time

            time.sleep(2.0)
    else:
        raise last_exc

    out_re = np.empty((BATCH, NFFT), np.float32)
    out_im = np.empty((BATCH, NFFT), np.float32)
    for c in range(NCORES):
        bsl = slice(c * BPC, (c + 1) * BPC)
        # dump[(h*8+g)*128+p, col]: p=(b2,kt,b0), col=(reim, j, ko);
        # b = h*256 + g*32 + j*8 + 2*b2 + b0, k = kt*128 + ko
        d = br.results[c]["odump"].reshape(NH, NGRPH, 4, N2, 2, 2, 4, 128)
        arr = (
            d.transpose(5, 0, 1, 6, 2, 4, 3, 7)
            .reshape(2, BPC, NFFT)
            .astype(np.float32)
        )
        out_re[bsl, :] = arr[0]
        out_im[bsl, :] = arr[1]
    return (out_re, out_im), br


def kernel(signal_re, signal_im):
    return run(signal_re, signal_im)[0]
